# revision 1
# baseline (speedup 1.0000x reference)
"""CrossAttention Trainium2 kernel (8-core SPMD, batch-parallel).

Problem: B=16, N=256, T=4096, D=1024, H=8, dh=128.
  q_n = LN(img_queries)*gamma_q ; x_n = LN(x)*gamma_x
  k = (x_n @ W_kv)[..., :128]  (single shared K head)
  sim = (q*scale) @ k^T ; attn = softmax(sim) ; out = attn @ x_n
Returns (out [B, N*H*D], attn [B,H,N,T]) like the reference.

Sharding: batch B across the 8 cores (2 per core), no collectives.
Matmul dtypes: fp32r (tf32-like, ~1.2e-4 rounding) for K/sim paths,
bf16 for the big attn@x_n matmul (lhsT=exp(sim^T), rhs=x_n).
softmax skips max-subtraction (logits are ~N(0, 0.65), exp is safe);
denominators come from the Exp activation's fp32 accum_out.
"""
import numpy as np
import concourse.bass as bass
import concourse.mybir as mybir
from concourse.tile import TileContext
from concourse.masks import make_identity
from concourse.bass_utils import run_bass_kernel_spmd

P = 128
B_FULL, N_Q, T_SEQ, D = 16, 256, 4096, 1024
HEADS, DH = 8, 128
N_CORES = 8
B_LOC = B_FULL // N_CORES          # 2 batches per core
SCALE = DH ** -0.5
EPS = 1e-5
F32 = mybir.dt.float32
F32R = mybir.dt.float32r
BF16 = mybir.dt.bfloat16
R_ROWS = HEADS * N_Q               # 2048 flattened (h, i) rows per batch
N_TCH = T_SEQ // P                 # 32 T-chunks of 128
N_TT = T_SEQ // 512                # 8 T-tiles of 512
N_RB = R_ROWS // 512               # 4 R-blocks of 512
AF = mybir.ActivationFunctionType


def _split_excess_waits(nc, max_waits=1):
    """Walrus encodes at most ONE sync-wait per instruction; move extra
    waits Tile emitted onto InstNoOp carriers on the same engine."""
    n = 0
    for fn in nc.m.functions:
        for bb in fn.blocks:
            out = []
            for inst in bb.instructions:
                si = inst.sync_info
                if (
                    si is not None
                    and si.on_wait
                    and len(si.on_wait) > max_waits
                    and not isinstance(inst, mybir.InstNoOp | mybir.InstEventSemaphore)
                ):
                    waits = list(si.on_wait)
                    for j, w in enumerate(waits[:-max_waits]):
                        out.append(mybir.InstNoOp(
                            name=f"{inst.name}-wn{j}",
                            engine=inst.engine,
                            sync_info=mybir.SyncInfo(on_wait=[w], on_update=[]),
                        ))
                    inst.sync_info = mybir.SyncInfo(
                        on_wait=waits[-max_waits:], on_update=list(si.on_update))
                    n += 1
                out.append(inst)
            bb.instructions = out
    return n


def _layer_norm_tile(nc, work, x_t, gamma_bc, eps_t):
    """x_t: [128, 1024] fp32 (modified in place), returns f32r normalized tile."""
    stats = work.tile([P, 2, 6], F32, tag="st", name="stats")
    nc.vector.bn_stats(out=stats[:, 0, :], in_=x_t[:, 0:512])
    nc.vector.bn_stats(out=stats[:, 1, :], in_=x_t[:, 512:1024])
    mv = work.tile([P, 2], F32, tag="mv", name="mv")
    nc.vector.bn_aggr(out=mv, in_=stats)
    rstd = work.tile([P, 1], F32, tag="rstd", name="rstd")
    nc.scalar.activation(out=rstd, in_=mv[:, 1:2], func=AF.Sqrt, bias=eps_t, scale=1.0)
    nc.vector.reciprocal(out=rstd, in_=rstd)
    nc.vector.tensor_scalar(
        out=x_t, in0=x_t, scalar1=mv[:, 0:1], scalar2=rstd,
        op0=mybir.AluOpType.subtract, op1=mybir.AluOpType.mult)
    xn_r = work.tile([P, 1024], F32R, tag="xnr", name="xn_r")
    nc.vector.tensor_mul(out=xn_r, in0=x_t, in1=gamma_bc)
    return xn_r


def build():
    nc = bass.Bass()
    iq = nc.dram_tensor("iq", [B_LOC, N_Q, D], F32, kind="ExternalInput")
    x = nc.dram_tensor("x", [B_LOC, T_SEQ, D], F32, kind="ExternalInput")
    gq = nc.dram_tensor("gq", [D], F32, kind="ExternalInput")
    gx = nc.dram_tensor("gx", [D], F32, kind="ExternalInput")
    wkv = nc.dram_tensor("wkv", [D, 2 * DH], F32, kind="ExternalInput")
    out_d = nc.dram_tensor("out", [B_LOC, N_Q, HEADS, D], F32, kind="ExternalOutput")
    attn_d = nc.dram_tensor("attn", [B_LOC, HEADS, N_Q, T_SEQ], F32, kind="ExternalOutput")

    with TileContext(nc) as tc:
        with tc.tile_pool(name="singles", bufs=1) as singles, \
             tc.tile_pool(name="resid", bufs=1) as resid, \
             tc.tile_pool(name="work", bufs=1) as work, \
             tc.tile_pool(name="att", bufs=1) as att:
            # ---- kernel-lifetime constants ----
            gq_bc = singles.tile([P, D], F32)
            nc.sync.dma_start(gq_bc, bass.AP(tensor=gq, offset=0, ap=[[0, P], [1, D]]))
            gx_bc = singles.tile([P, D], F32)
            nc.sync.dma_start(gx_bc, bass.AP(tensor=gx, offset=0, ap=[[0, P], [1, D]]))
            eps_t = singles.tile([P, 1], F32)
            nc.vector.memset(eps_t, EPS)
            ident_f32 = singles.tile([P, P], F32)
            make_identity(nc, ident_f32)
            ident_r = singles.tile([P, P], F32R)
            nc.vector.tensor_copy(ident_r, ident_f32)
            wk_f = singles.tile([P, 8, DH], F32)
            nc.sync.dma_start(
                wk_f, wkv.rearrange("(dc p) c -> p dc c", p=P)[:, :, 0:DH])
            wk_r = singles.tile([P, 8, DH], F32R)
            nc.vector.tensor_copy(wk_r, wk_f)

            for b in range(B_LOC):
                # ================= Phase 1+2: LN + K^T + Q^T =================
                kt = resid.tile([P, T_SEQ], F32R, tag="kt", name="kt")
                qt = resid.tile([P, R_ROWS], F32R, tag="qt", name="qt")
                xbf = [resid.tile([P, D], BF16, tag=f"xbf{tch}", name=f"xbf{tch}")
                       for tch in range(N_TCH)]
                sinv_all = resid.tile([P, 16], F32, tag="sinv", name="sinv_all")

                with tc.tile_pool(name="ps1", bufs=1, space="PSUM") as ps1:
                    for tt in range(N_TT):
                        xnt = work.tile([P, 8, 512], F32R, tag="xnt", name="xnt")
                        for tci in range(4):
                            tch = tt * 4 + tci
                            x_t = work.tile([P, D], F32, tag="xt", bufs=2, name="x_t")
                            nc.sync.dma_start(x_t, x[b, tch * P:(tch + 1) * P, :])
                            xn_r = _layer_norm_tile(nc, work, x_t, gx_bc, eps_t)
                            nc.gpsimd.tensor_copy(out=xbf[tch], in_=xn_r)
                            for dc in range(8):
                                ps_t = ps1.tile([P, P], F32, tag="pst", bufs=2,
                                                name="ps_t")
                                nc.tensor.transpose(
                                    ps_t.bitcast(F32R),
                                    xn_r[:, dc * P:(dc + 1) * P], ident_r)
                                nc.vector.tensor_copy(
                                    out=xnt[:, dc, tci * P:(tci + 1) * P],
                                    in_=ps_t.bitcast(F32R))
                        ps_kt = ps1.tile([P, 512], F32, tag="kt", bufs=2, name="ps_kt")
                        for dc in range(8):
                            nc.tensor.matmul(ps_kt, wk_r[:, dc, :], xnt[:, dc, :],
                                             start=(dc == 0), stop=(dc == 7))
                        nc.vector.tensor_copy(out=kt[:, tt * 512:(tt + 1) * 512],
                                              in_=ps_kt)

                    # Q: LN + per-head transpose into qt [dh, h*256+i]
                    for i2 in range(2):
                        q_t = work.tile([P, D], F32, tag="xt", bufs=2, name="q_t")
                        nc.sync.dma_start(q_t, iq[b, i2 * P:(i2 + 1) * P, :])
                        qn_r = _layer_norm_tile(nc, work, q_t, gq_bc, eps_t)
                        for h in range(HEADS):
                            ps_q = ps1.tile([P, P], F32, tag="pst", bufs=2, name="ps_q")
                            nc.tensor.transpose(
                                ps_q.bitcast(F32R),
                                qn_r[:, h * DH:(h + 1) * DH], ident_r)
                            nc.vector.tensor_copy(
                                out=qt[:, h * N_Q + i2 * P: h * N_Q + (i2 + 1) * P],
                                in_=ps_q.bitcast(F32R))

                # ================= Phase 3: attention =================
                with tc.tile_pool(name="ps3", bufs=1, space="PSUM") as ps3:
                    for rb in range(N_RB):
                        # --- (a) natural-layout passes: S, then attn write ---
                        for rc in range(4):
                            ri = rb * 4 + rc
                            r0 = ri * P
                            h, i0 = r0 // N_Q, r0 % N_Q
                            ssub = att.tile([P, 8], F32, tag="ssub", bufs=2,
                                            name="ssub")
                            for tt in range(N_TT):
                                ps_nat = ps3.tile([P, 512], F32, tag="nat", bufs=2,
                                                  name="ps_nat")
                                nc.tensor.matmul(ps_nat, qt[:, r0:r0 + P],
                                                 kt[:, tt * 512:(tt + 1) * 512],
                                                 start=True, stop=True)
                                e_scr = att.tile([P, 512], F32, tag="escr", bufs=2,
                                                 name="e_scr")
                                nc.scalar.activation(
                                    out=e_scr, in_=ps_nat, func=AF.Exp, scale=SCALE,
                                    accum_out=ssub[:, tt:tt + 1])
                            s_col = att.tile([P, 1], F32, tag="scol", bufs=2,
                                             name="s_col")
                            nc.vector.reduce_sum(out=s_col, in_=ssub,
                                                 axis=mybir.AxisListType.X)
                            nc.vector.reciprocal(out=sinv_all[:, ri:ri + 1], in_=s_col)
                            # second pass: recompute, normalize, write attn
                            for tt in range(N_TT):
                                ps_nat2 = ps3.tile([P, 512], F32, tag="nat", bufs=2,
                                                   name="ps_nat2")
                                nc.tensor.matmul(ps_nat2, qt[:, r0:r0 + P],
                                                 kt[:, tt * 512:(tt + 1) * 512],
                                                 start=True, stop=True)
                                e_t = att.tile([P, 512], F32, tag="escr", bufs=2,
                                               name="e_t")
                                nc.scalar.activation(out=e_t, in_=ps_nat2,
                                                     func=AF.Exp, scale=SCALE)
                                a_sb = att.tile([P, 512], F32, tag="asb", bufs=3,
                                                name="a_sb")
                                nc.vector.tensor_scalar_mul(
                                    out=a_sb, in0=e_t, scalar1=sinv_all[:, ri:ri + 1])
                                nc.sync.dma_start(
                                    attn_d[b, h, i0:i0 + P, tt * 512:(tt + 1) * 512],
                                    a_sb)

                        # --- (b) sim^T sweep -> E^T (bf16) ---
                        ets = []
                        for tch in range(N_TCH):
                            ps_st = ps3.tile([P, 512], F32, tag="sT", bufs=2,
                                             name="ps_st")
                            nc.tensor.matmul(ps_st, kt[:, tch * P:(tch + 1) * P],
                                             qt[:, rb * 512:(rb + 1) * 512],
                                             start=True, stop=True)
                            et = att.tile([P, 512], BF16, tag="et", bufs=34, name="et")
                            nc.scalar.activation(out=et, in_=ps_st, func=AF.Exp,
                                                 scale=SCALE)
                            ets.append(et)

                        # --- (c) out matmul + normalize ---
                        for rc in range(4):
                            ri = rb * 4 + rc
                            r0 = ri * P
                            h, i0 = r0 // N_Q, r0 % N_Q
                            ps_o0 = ps3.tile([P, 512], F32, tag="o", bufs=4,
                                             name="ps_o0")
                            ps_o1 = ps3.tile([P, 512], F32, tag="o", bufs=4,
                                             name="ps_o1")
                            for tch in range(N_TCH):
                                lhs = ets[tch][:, rc * P:(rc + 1) * P]
                                nc.tensor.matmul(ps_o0, lhs, xbf[tch][:, 0:512],
                                                 start=(tch == 0), stop=(tch == 31))
                                nc.tensor.matmul(ps_o1, lhs, xbf[tch][:, 512:1024],
                                                 start=(tch == 0), stop=(tch == 31))
                            out_sb = att.tile([P, D], F32, tag="osb", bufs=2,
                                              name="out_sb")
                            nc.vector.tensor_scalar_mul(
                                out=out_sb[:, 0:512], in0=ps_o0,
                                scalar1=sinv_all[:, ri:ri + 1])
                            nc.vector.tensor_scalar_mul(
                                out=out_sb[:, 512:1024], in0=ps_o1,
                                scalar1=sinv_all[:, ri:ri + 1])
                            nc.sync.dma_start(out_d[b, i0:i0 + P, h, :], out_sb)

    _split_excess_waits(nc)
    return nc


_NC_CACHE = None


def _get_nc():
    global _NC_CACHE
    if _NC_CACHE is None:
        _NC_CACHE = build()
    return _NC_CACHE


def run(inputs, trace=False):
    nc = _get_nc()
    in_maps = []
    for c in range(N_CORES):
        b0 = c * B_LOC
        in_maps.append({
            "iq": np.ascontiguousarray(inputs["img_queries"][b0:b0 + B_LOC]),
            "x": np.ascontiguousarray(inputs["x"][b0:b0 + B_LOC]),
            "gq": np.asarray(inputs["gamma_q"]),
            "gx": np.asarray(inputs["gamma_x"]),
            "wkv": np.asarray(inputs["W_kv"]),
        })
    res = run_bass_kernel_spmd(nc, in_maps, list(range(N_CORES)), trace=trace)
    out = np.concatenate(
        [r["out"].reshape(B_LOC, N_Q * HEADS * D) for r in res.results], axis=0)
    attn = np.concatenate([r["attn"] for r in res.results], axis=0)
    return (out, attn), res


def kernel(**inputs):
    (out, attn), _ = run(inputs)
    return out, attn
